# revision 14
# baseline (speedup 1.0000x reference)
"""Trainium2 Bass kernel for a 2-layer cross-attention dense transformer.

Sharding: data-parallel over batch — B=8 batch elements, one per NeuronCore.
Each core runs the full 2-layer transformer on its batch element; weights are
replicated. No collectives.

Per-core layout choices:
  - Activations are kept FEATURE-major in SBUF: x_fm[p, blk, l] = x[blk*128+p, l].
    All matmuls contract over features, so no transposes are needed anywhere.
  - Attention scores are computed transposed (scores_T[k_pos, q]); softmax
    denominators are obtained by appending a ones-column to the V tile in the
    attn@V matmul (M=65 per head), then dividing at PSUM eviction.
  - No max-subtraction in softmax (scores are O(5); exp is safe in fp32).
  - Matmuls run in float32r (full-rate fp32, ~TF32 input precision). PSUM fp32.
  - LayerNorm partition-dim sums via ones-vector matmuls; rsqrt via
    exp(-0.5*ln(v+eps)) so the ACT table never leaves the ln/exp set during
    attention (gelu is the only table switch).
"""

import numpy as np

# ---------------------------------------------------------------- constants
B, D, L0 = 8, 512, 1024
L = L0 + 1            # 1025 tokens (cls + 1024)
H, DH = 8, 64
DFF = 2048
NLAYER = 2
EPS = 1e-6
SCALE = 1.0 / (DH ** 0.5)

P = 128
DB = D // P           # 4 feature blocks
FB = DFF // P         # 16 dff blocks
LB = (L + P - 1) // P  # 9 l-tiles (8 full + 1 single-row)

N_CORES = 8


def _chunks(total, width):
    out, o = [], 0
    while o < total:
        w = min(width, total - o)
        out.append((o, w))
        o += w
    return out


QC_ATT = _chunks(L, 256)   # attention q-chunks (fp32r wants N>=256)
QC_FFN = _chunks(L, 512)   # FFN l-chunks
NT_PROJ = _chunks(L, 512)  # N-tiling for full-L projections (K, V)

# params tile slot indices (free-dim column j of the [128, NPARAM] tile)
BQ, BK, BV, BO, B2S = 0, 4, 8, 12, 16
LN1G, LN1B, LN2G, LN2B = 20, 24, 28, 32
B1S = 36
NPARAM = 52

_CACHE = {}


# ---------------------------------------------------------------- bass build
def _build_nc():
    import concourse.bass as bass
    import concourse.bacc as bacc
    import concourse.tile as tile
    from concourse import mybir
    from concourse.masks import make_identity

    f32 = mybir.dt.float32
    f32r = mybir.dt.float32r
    AO = mybir.AluOpType
    AF = mybir.ActivationFunctionType

    nc = bacc.Bacc("TRN2", target_bir_lowering=False, debug=False)

    # ---- DRAM I/O (per core) ----
    e1 = nc.dram_tensor("e1", [D, L0], f32, kind="ExternalInput")
    e2 = nc.dram_tensor("e2", [D, L0], f32, kind="ExternalInput")
    cls_t = nc.dram_tensor("cls", [D, 1], f32, kind="ExternalInput")
    wqT = nc.dram_tensor("wqT", [NLAYER, D, D], f32, kind="ExternalInput")
    wkT = nc.dram_tensor("wkT", [NLAYER, D, D], f32, kind="ExternalInput")
    wvT = nc.dram_tensor("wvT", [NLAYER, D, D], f32, kind="ExternalInput")
    woTh = nc.dram_tensor("woTh", [NLAYER, DH, H, D], f32, kind="ExternalInput")
    w1T = nc.dram_tensor("w1T", [NLAYER, D, DFF], f32, kind="ExternalInput")
    w2T = nc.dram_tensor("w2T", [NLAYER, DFF, D], f32, kind="ExternalInput")
    params_d = nc.dram_tensor("params", [NLAYER, P, NPARAM], f32,
                              kind="ExternalInput")
    bvrow_d = nc.dram_tensor("bvrow", [NLAYER, D], f32, kind="ExternalInput")
    out_d = nc.dram_tensor("out", [L, D], f32, kind="ExternalOutput")

    def r(ap):
        return ap.bitcast(f32r)

    def mm(out, lhsT, rhs, start, stop, n):
        # fp32r matmuls fail ISA checks for tiny moving operands; fall back
        # to plain fp32 (4 cyc/row, negligible at n==1) for edge chunks.
        if n == 1:
            nc.tensor.matmul(out, lhsT.bitcast(f32), rhs.bitcast(f32),
                             start=start, stop=stop)
        else:
            nc.tensor.matmul(out, lhsT, rhs, start=start, stop=stop)

    def layernorm(psp, pstag, rows, sqpool, sqtag, sqshape, src, qw, prms,
                  GSLOT, BSLOT, out_ap, ones_col, ones_row):
        """LN over features of src [P, DB, qw] -> out_ap [P, DB, qw]."""
        # partition sums via ones-matmuls
        mp = psp.tile([P, 512], f32, tag=pstag)
        for kt in range(DB):
            mm(mp[0:1, :qw], ones_col[:, 0:1], src[:, kt, :qw],
               (kt == 0), (kt == DB - 1), qw)
        sq = sqpool.tile(sqshape, f32r, tag=sqtag)
        nc.vector.tensor_mul(sq[:, :, :qw], src[:, :, :qw], src[:, :, :qw])
        sp = psp.tile([P, 512], f32, tag=pstag)
        for kt in range(DB):
            mm(sp[0:1, :qw], ones_col[:, 0:1], sq[:, kt, :qw],
               (kt == 0), (kt == DB - 1), qw)
        # mean row (f32r so it can feed the broadcast matmul) + its bcast
        m_row = rows.tile([1, 512], f32r, tag="row")
        nc.vector.tensor_scalar_mul(m_row[:, :qw], mp[0:1, :qw], 1.0 / D)
        mb = psp.tile([P, 512], f32, tag=pstag)
        mm(mb[:, :qw], ones_row[0:1, :], m_row[0:1, :qw], True, True, qw)
        # var = sq/D - mean^2 ; rstd = exp(-0.5*ln(var+eps))
        msq = rows.tile([1, 512], f32, tag="row")
        nc.vector.tensor_mul(msq[:, :qw], m_row[:, :qw], m_row[:, :qw])
        v_row = rows.tile([1, 512], f32, tag="row")
        nc.vector.scalar_tensor_tensor(
            out=v_row[:, :qw], in0=sp[0:1, :qw], scalar=1.0 / D,
            in1=msq[:, :qw], op0=AO.mult, op1=AO.subtract)
        lnv = rows.tile([1, 512], f32, tag="row")
        nc.scalar.activation(lnv[:, :qw], v_row[:, :qw], AF.Ln, bias=eps_row[:])
        nc.vector.tensor_scalar_mul(lnv[:, :qw], lnv[:, :qw], -0.5)
        r_row = rows.tile([1, 512], f32r, tag="row")
        nc.scalar.activation(r_row[:, :qw], lnv[:, :qw], AF.Exp)
        rb = psp.tile([P, 512], f32, tag=pstag)
        mm(rb[:, :qw], ones_row[0:1, :], r_row[0:1, :qw], True, True, qw)
        # apply per block: out = ((src - mb) * rb) * g + b
        for m in range(DB):
            nc.vector.tensor_sub(sq[:, m, :qw], src[:, m, :qw], mb[:, :qw])
            nc.vector.tensor_mul(sq[:, m, :qw], sq[:, m, :qw], rb[:, :qw])
            nc.scalar.activation(
                out_ap[:, m, :], sq[:, m, :qw], AF.Identity,
                bias=prms[:, BSLOT + m:BSLOT + m + 1],
                scale=prms[:, GSLOT + m:GSLOT + m + 1])

    with tile.TileContext(nc) as tc:
        with tc.tile_pool(name="persist", bufs=1) as pp, \
             tc.tile_pool(name="xpool", bufs=1) as xp, \
             tc.tile_pool(name="parms", bufs=2) as prm_pool:

            ones_f32 = pp.tile([P, P], f32)
            nc.vector.memset(ones_f32[:], 1.0)
            ones_col = pp.tile([P, 1], f32r)
            nc.vector.tensor_copy(ones_col[:], ones_f32[:, 0:1])
            ones_row = pp.tile([1, P], f32r)
            nc.vector.tensor_copy(ones_row[:], ones_f32[0:1, :])
            ident = pp.tile([P, P], f32)
            make_identity(nc, ident[:])
            eps_row = pp.tile([1, 1], f32)
            nc.vector.memset(eps_row[:], EPS)

            x = xp.tile([P, DB, L], f32r, tag="x")
            x2 = xp.tile([P, DB, L], f32r, tag="x2")
            xmid = xp.tile([P, DB, L], f32r, tag="xmid")

            for m in range(DB):
                nc.sync.dma_start(x[:, m, 1:L], r(e1[m * P:(m + 1) * P, :]))
                nc.sync.dma_start(x2[:, m, 1:L], r(e2[m * P:(m + 1) * P, :]))
                nc.sync.dma_start(x[:, m, 0:1], r(cls_t[m * P:(m + 1) * P, :]))
                nc.sync.dma_start(x2[:, m, 0:1], r(cls_t[m * P:(m + 1) * P, :]))

            for l in range(NLAYER):
                prms = prm_pool.tile([P, NPARAM], f32, tag="prms")
                nc.sync.dma_start(prms[:], params_d[l, :, :])

                # =================== PHASE A: attention ===================
                with tc.tile_pool(name=f"wA{l}", bufs=2) as wp, \
                     tc.tile_pool(name=f"kvA{l}", bufs=1) as ap1, \
                     tc.tile_pool(name=f"dbA{l}", bufs=2) as ap2, \
                     tc.tile_pool(name=f"sbA{l}", bufs=1) as ap3, \
                     tc.tile_pool(name=f"rwA{l}", bufs=6) as rows, \
                     tc.tile_pool(name=f"psA{l}", bufs=4, space="PSUM") as psA, \
                     tc.tile_pool(name=f"psC{l}", bufs=2, space="PSUM") as psC, \
                     tc.tile_pool(name=f"psW{l}", bufs=2, space="PSUM") as psW:

                    # ---- K projection (full L) ----
                    wk_sb = wp.tile([P, DB, D], f32r, tag="w")
                    nc.sync.dma_start(
                        wk_sb[:],
                        r(wkT[l, :, :].rearrange("(b p) n -> p b n", p=P)))
                    K_fm = ap1.tile([P, DB, L], f32r, tag="K")
                    for m in range(DB):
                        for (o, w) in NT_PROJ:
                            kp = psA.tile([P, 512], f32, tag="psA")
                            for kt in range(DB):
                                mm(kp[:, :w],
                                   wk_sb[:, kt, m * P:(m + 1) * P],
                                   x2[:, kt, o:o + w],
                                   (kt == 0), (kt == DB - 1), w)
                            nc.vector.tensor_scalar(
                                out=K_fm[:, m, o:o + w], in0=kp[:, :w],
                                scalar1=prms[:, BK + m:BK + m + 1],
                                scalar2=SCALE, op0=AO.add, op1=AO.mult)

                    # ---- V projection (token-major, ones column at DH) ----
                    wv_sb = wp.tile([P, DB, D], f32r, tag="w")
                    nc.sync.dma_start(
                        wv_sb[:],
                        r(wvT[l, :, :].rearrange("(b p) n -> p b n", p=P)))
                    bvb = ap1.tile([P, D], f32, tag="bvb")
                    nc.sync.dma_start(
                        bvb[:],
                        bass.AP(tensor=bvrow_d, offset=l * D,
                                ap=[[0, P], [1, D]]))
                    V_tm = ap1.tile([P, LB, H, DH + 1], f32r, tag="V")
                    nc.vector.tensor_copy(
                        V_tm[:, :, :, DH],
                        ones_f32[:, 0:LB * H].rearrange(
                            "p (a b) -> p a b", a=LB))
                    for mt in range(LB):
                        nrow = P if mt < LB - 1 else L - (LB - 1) * P
                        vp = psA.tile([P, 512], f32, tag="psA")
                        for kt in range(DB):
                            nc.tensor.matmul(
                                vp[:nrow, :D],
                                x2[:, kt, mt * P:mt * P + nrow],
                                wv_sb[:, kt, :],
                                start=(kt == 0), stop=(kt == DB - 1))
                        nc.vector.tensor_tensor(
                            out=V_tm[:nrow, mt, :, 0:DH],
                            in0=vp[:nrow, :D].rearrange("p (h c) -> p h c", h=H),
                            in1=bvb[:nrow, :].rearrange("p (h c) -> p h c", h=H),
                            op=AO.add)

                    # ---- wq & wo loads ----
                    wq_sb = wp.tile([P, DB, D], f32r, tag="w")
                    nc.sync.dma_start(
                        wq_sb[:],
                        r(wqT[l, :, :].rearrange("(b p) n -> p b n", p=P)))
                    wo_sb = wp.tile([DH, H, D], f32r, tag="w")
                    nc.sync.dma_start(wo_sb[:], r(woTh[l, :, :, :]))

                    # ---- per q-chunk attention ----
                    for (qo, qw) in QC_ATT:
                        Q_fm = ap3.tile([P, DB, 256], f32r, tag="Q")
                        for m in range(DB):
                            qp = psA.tile([P, 512], f32, tag="psA")
                            for kt in range(DB):
                                mm(qp[:, :qw],
                                   wq_sb[:, kt, m * P:(m + 1) * P],
                                   x[:, kt, qo:qo + qw],
                                   (kt == 0), (kt == DB - 1), qw)
                            nc.vector.tensor_scalar_add(
                                Q_fm[:, m, :qw], qp[:, :qw],
                                prms[:, BQ + m:BQ + m + 1])

                        ctx_sb = ap3.tile([DH, H, 256], f32r, tag="ctx")

                        # pipeline scores/exp one head ahead of attn@V
                        exp_tiles = {}
                        for h in range(H + 1):
                            if h < H:
                                base, blk = (h % 2) * DH, h // 2
                                et = ap2.tile([P, LB, 256], f32r, tag="exp")
                                for kt in range(LB):
                                    nrow = P if kt < LB - 1 else L - (LB - 1) * P
                                    sp = psA.tile([P, 512], f32, tag="psA")
                                    mm(sp[:nrow, :qw],
                                       K_fm[base:base + DH, blk,
                                            kt * P:kt * P + nrow],
                                       Q_fm[base:base + DH, blk, :qw],
                                       True, True, qw)
                                    nc.scalar.activation(
                                        et[:nrow, kt, :qw], sp[:nrow, :qw],
                                        AF.Exp)
                                exp_tiles[h] = et
                            if h > 0:
                                hh = h - 1
                                et = exp_tiles.pop(hh)
                                cp = psC.tile([DH + 1, 256], f32, tag="psC")
                                for kt in range(LB):
                                    nrow = P if kt < LB - 1 else L - (LB - 1) * P
                                    mm(cp[:, :qw],
                                       V_tm[:nrow, kt, hh, :],
                                       et[:nrow, kt, :qw],
                                       (kt == 0), (kt == LB - 1), qw)
                                rrow = rows.tile([1, 512], f32, tag="row")
                                nc.vector.reciprocal(rrow[:, :qw],
                                                     cp[DH:DH + 1, :qw])
                                rrowr = rows.tile([1, 512], f32r, tag="row")
                                nc.vector.tensor_copy(rrowr[:, :qw],
                                                      rrow[:, :qw])
                                rb = psA.tile([P, 512], f32, tag="psA")
                                mm(rb[:DH, :qw], ones_row[0:1, 0:DH],
                                   rrowr[0:1, :qw], True, True, qw)
                                nc.vector.tensor_copy(ctx_sb[:, hh, :qw],
                                                      cp[:DH, :qw])
                                nc.vector.tensor_tensor(
                                    out=ctx_sb[:, hh, :qw],
                                    in0=ctx_sb[:, hh, :qw],
                                    in1=rb[:DH, :qw], op=AO.mult)

                        # ---- output projection + bo + residual ----
                        xa = ap3.tile([P, DB, 256], f32r, tag="xa")
                        for m in range(DB):
                            op_ = psW.tile([P, 512], f32, tag="psW")
                            for h in range(H):
                                mm(op_[:, :qw],
                                   wo_sb[:, h, m * P:(m + 1) * P],
                                   ctx_sb[:, h, :qw],
                                   (h == 0), (h == H - 1), qw)
                            nc.vector.scalar_tensor_tensor(
                                out=xa[:, m, :qw], in0=op_[:, :qw],
                                scalar=prms[:, BO + m:BO + m + 1],
                                in1=x[:, m, qo:qo + qw],
                                op0=AO.add, op1=AO.add)

                        layernorm(psA, "psA", rows, ap2, "sq", [P, DB, 256],
                                  xa, qw, prms, LN1G, LN1B,
                                  xmid[:, :, qo:qo + qw], ones_col, ones_row)

                # =================== PHASE B: FFN ===================
                with tc.tile_pool(name=f"wB{l}", bufs=1) as fwp, \
                     tc.tile_pool(name=f"hB{l}", bufs=1) as fhp, \
                     tc.tile_pool(name=f"xB{l}", bufs=2) as fxp, \
                     tc.tile_pool(name=f"rwB{l}", bufs=6) as rowsB, \
                     tc.tile_pool(name=f"psH{l}", bufs=4, space="PSUM") as psH, \
                     tc.tile_pool(name=f"psF{l}", bufs=2, space="PSUM") as psF:

                    w1_sb = fwp.tile([P, DB, DFF], f32r, tag="w1")
                    nc.sync.dma_start(
                        w1_sb[:],
                        r(w1T[l, :, :].rearrange("(b p) n -> p b n", p=P)))
                    w2_sb = fwp.tile([P, FB, D], f32r, tag="w2")
                    nc.sync.dma_start(
                        w2_sb[:],
                        r(w2T[l, :, :].rearrange("(b p) n -> p b n", p=P)))

                    for (qo, qw) in QC_FFN:
                        h_sb = fhp.tile([P, FB, 512], f32r, tag="h")
                        for mf in range(FB):
                            hp = psH.tile([P, 512], f32, tag="psH")
                            for kt in range(DB):
                                mm(hp[:, :qw],
                                   w1_sb[:, kt, mf * P:(mf + 1) * P],
                                   xmid[:, kt, qo:qo + qw],
                                   (kt == 0), (kt == DB - 1), qw)
                            nc.scalar.activation(
                                h_sb[:, mf, :qw], hp[:, :qw], AF.Gelu,
                                bias=prms[:, B1S + mf:B1S + mf + 1])
                        xb = fxp.tile([P, DB, 512], f32r, tag="xb")
                        for m in range(DB):
                            fp = psF.tile([P, 512], f32, tag="psF")
                            for kt in range(FB):
                                mm(fp[:, :qw],
                                   w2_sb[:, kt, m * P:(m + 1) * P],
                                   h_sb[:, kt, :qw],
                                   (kt == 0), (kt == FB - 1), qw)
                            nc.vector.scalar_tensor_tensor(
                                out=xb[:, m, :qw], in0=fp[:, :qw],
                                scalar=prms[:, B2S + m:B2S + m + 1],
                                in1=xmid[:, m, qo:qo + qw],
                                op0=AO.add, op1=AO.add)
                        layernorm(psH, "psH", rowsB, fxp, "sq", [P, DB, 512],
                                  xb, qw, prms, LN2G, LN2B,
                                  x[:, :, qo:qo + qw], ones_col, ones_row)

            # =================== transpose x -> out ===================
            with tc.tile_pool(name="psT", bufs=4, space="PSUM") as psT, \
                 tc.tile_pool(name="sbT", bufs=4) as sbT:
                for mt in range(LB):
                    nrow = P if mt < LB - 1 else L - (LB - 1) * P
                    for m in range(DB):
                        tp = psT.tile([P, P], f32, tag="psT")
                        nc.tensor.transpose(
                            tp[:nrow, :],
                            x[:, m, mt * P:mt * P + nrow].bitcast(f32),
                            ident[:])
                        ts = sbT.tile([P, P], f32, tag="sbT")
                        nc.vector.tensor_copy(ts[:nrow, :], tp[:nrow, :])
                        nc.sync.dma_start(
                            out_d[mt * P:mt * P + nrow, m * P:(m + 1) * P],
                            ts[:nrow, :])

    nc.compile()
    return nc


# ---------------------------------------------------------------- host side
def _prep_inputs(inputs):
    f = np.float32
    Wq, Wk, Wv, Wo = inputs["Wq"], inputs["Wk"], inputs["Wv"], inputs["Wo"]
    W1, W2 = inputs["W1"], inputs["W2"]

    wqT = np.ascontiguousarray(np.transpose(np.asarray(Wq, f), (0, 2, 1)))
    wkT = np.ascontiguousarray(np.transpose(np.asarray(Wk, f), (0, 2, 1)))
    wvT = np.ascontiguousarray(np.transpose(np.asarray(Wv, f), (0, 2, 1)))
    w1T = np.ascontiguousarray(np.transpose(np.asarray(W1, f), (0, 2, 1)))
    w2T = np.ascontiguousarray(np.transpose(np.asarray(W2, f), (0, 2, 1)))
    woTh = np.ascontiguousarray(
        np.transpose(np.asarray(Wo, f).reshape(NLAYER, D, H, DH), (0, 3, 2, 1)))

    def col(v):  # [NLAYER, D] -> [NLAYER, P, DB]
        return np.transpose(np.asarray(v, f).reshape(NLAYER, DB, P), (0, 2, 1))

    params = np.zeros((NLAYER, P, NPARAM), f)
    params[:, :, BQ:BQ + DB] = col(inputs["bq"])
    params[:, :, BK:BK + DB] = col(inputs["bk"])
    params[:, :, BV:BV + DB] = col(inputs["bv"])
    params[:, :, BO:BO + DB] = col(inputs["bo"])
    params[:, :, B2S:B2S + DB] = col(inputs["b2"])
    params[:, :, LN1G:LN1G + DB] = col(inputs["ln1_g"])
    params[:, :, LN1B:LN1B + DB] = col(inputs["ln1_b"])
    params[:, :, LN2G:LN2G + DB] = col(inputs["ln2_g"])
    params[:, :, LN2B:LN2B + DB] = col(inputs["ln2_b"])
    params[:, :, B1S:B1S + FB] = np.transpose(
        np.asarray(inputs["b1"], f).reshape(NLAYER, FB, P), (0, 2, 1))

    shared = {
        "cls": np.ascontiguousarray(
            np.asarray(inputs["cls_token"], f).reshape(D, 1)),
        "wqT": wqT, "wkT": wkT, "wvT": wvT, "woTh": woTh,
        "w1T": w1T, "w2T": w2T, "params": params,
        "bvrow": np.ascontiguousarray(np.asarray(inputs["bv"], f)),
    }
    e1 = np.asarray(inputs["embed1"], f)
    e2 = np.asarray(inputs["embed2"], f)
    in_maps = []
    for b in range(N_CORES):
        m = dict(shared)
        m["e1"] = np.ascontiguousarray(e1[b])
        m["e2"] = np.ascontiguousarray(e2[b])
        in_maps.append(m)
    return in_maps


def _run(inputs, trace=False, **kw):
    from concourse.bass_utils import run_bass_kernel_spmd

    if "nc" not in _CACHE:
        _CACHE["nc"] = _build_nc()
    nc = _CACHE["nc"]
    in_maps = _prep_inputs(inputs)
    res = run_bass_kernel_spmd(nc, in_maps, list(range(N_CORES)), trace=trace,
                               **kw)
    out = np.stack([res.results[b]["out"] for b in range(N_CORES)], axis=0)
    return out.astype(np.float32), res


def kernel(**inputs):
    out, _ = _run(inputs, trace=False)
    return out


# revision 16
# speedup vs baseline: 1.1048x; 1.1048x over previous
"""Trainium2 Bass kernel for a 2-layer cross-attention dense transformer.

Sharding: data-parallel over batch — B=8 batch elements, one per NeuronCore.
Each core runs the full 2-layer transformer on its batch element; weights are
replicated. No collectives.

Per-core layout choices:
  - Activations are kept FEATURE-major in SBUF: x_fm[p, blk, l] = x[blk*128+p, l].
    All matmuls contract over features, so no transposes are needed anywhere.
  - Attention scores are computed transposed (scores_T[k_pos, q]); softmax
    denominators are obtained by appending a ones-column to the V tile in the
    attn@V matmul (M=65 per head), then dividing at PSUM eviction.
  - No max-subtraction in softmax (scores are O(5); exp is safe in fp32).
  - Matmuls run in float32r (single-pass fp32; ~TF32 input rounding). PSUM fp32.
  - LayerNorm partition sums via ones-vector matmuls, accumulated to full-L
    row tiles; rsqrt = exp(-0.5*ln(v+eps)) ONCE per layer-norm site so the
    ACT table only reloads a few times per layer.
"""

import numpy as np

# ---------------------------------------------------------------- constants
B, D, L0 = 8, 512, 1024
L = L0 + 1            # 1025 tokens (cls + 1024)
H, DH = 8, 64
DFF = 2048
NLAYER = 2
EPS = 1e-6
SCALE = 1.0 / (DH ** 0.5)

P = 128
DB = D // P           # 4 feature blocks
FB = DFF // P         # 16 dff blocks
LB = (L + P - 1) // P  # 9 l-tiles (8 full + 1 single-row)

N_CORES = 8


def _chunks(total, width):
    out, o = [], 0
    while o < total:
        w = min(width, total - o)
        out.append((o, w))
        o += w
    return out


QC = _chunks(L, 512)       # [(0,512),(512,512),(1024,1)]

# params tile slot indices (free-dim column j of the [128, NPARAM] tile)
BQ, BK, BV, BO, B2S = 0, 4, 8, 12, 16
LN1G, LN1B, LN2G, LN2B = 20, 24, 28, 32
B1S = 36
NPARAM = 52

_CACHE = {}


# ---------------------------------------------------------------- bass build
def _build_nc():
    import concourse.bass as bass
    import concourse.bacc as bacc
    import concourse.tile as tile
    from concourse import mybir
    from concourse.masks import make_identity

    f32 = mybir.dt.float32
    f32r = mybir.dt.float32r
    AO = mybir.AluOpType
    AF = mybir.ActivationFunctionType

    nc = bacc.Bacc("TRN2", target_bir_lowering=False, debug=False)

    # ---- DRAM I/O (per core) ----
    e1 = nc.dram_tensor("e1", [D, L0], f32, kind="ExternalInput")
    e2 = nc.dram_tensor("e2", [D, L0], f32, kind="ExternalInput")
    cls_t = nc.dram_tensor("cls", [D, 1], f32, kind="ExternalInput")
    wqT = nc.dram_tensor("wqT", [NLAYER, D, D], f32, kind="ExternalInput")
    wkT = nc.dram_tensor("wkT", [NLAYER, D, D], f32, kind="ExternalInput")
    wvT = nc.dram_tensor("wvT", [NLAYER, D, D], f32, kind="ExternalInput")
    woTh = nc.dram_tensor("woTh", [NLAYER, DH, H, D], f32, kind="ExternalInput")
    w1T = nc.dram_tensor("w1T", [NLAYER, D, DFF], f32, kind="ExternalInput")
    w2T = nc.dram_tensor("w2T", [NLAYER, DFF, D], f32, kind="ExternalInput")
    params_d = nc.dram_tensor("params", [NLAYER, P, NPARAM], f32,
                              kind="ExternalInput")
    bvrow_d = nc.dram_tensor("bvrow", [NLAYER, D], f32, kind="ExternalInput")
    out_d = nc.dram_tensor("out", [L, D], f32, kind="ExternalOutput")

    def r(ap):
        return ap.bitcast(f32r)

    def mm(out, lhsT, rhs, start, stop, n):
        # fp32r matmuls fail ISA checks for 1-wide moving operands; fall back
        # to plain fp32 there (edge chunks only).
        if n == 1:
            nc.tensor.matmul(out, lhsT.bitcast(f32), rhs.bitcast(f32),
                             start=start, stop=stop)
        else:
            nc.tensor.matmul(out, lhsT, rhs, start=start, stop=stop)

    with tile.TileContext(nc) as tc:
        with tc.tile_pool(name="persist", bufs=1) as pp, \
             tc.tile_pool(name="xpool", bufs=1) as xp, \
             tc.tile_pool(name="parms", bufs=2) as prm_pool:

            ones_f32 = pp.tile([P, P], f32)
            nc.vector.memset(ones_f32[:], 1.0)
            ones_col = pp.tile([P, 1], f32r)
            nc.vector.tensor_copy(ones_col[:], ones_f32[:, 0:1])
            ones_row = pp.tile([1, P], f32r)
            nc.vector.tensor_copy(ones_row[:], ones_f32[0:1, :])
            ident = pp.tile([P, P], f32)
            make_identity(nc, ident[:])
            eps_row = pp.tile([1, 1], f32)
            nc.vector.memset(eps_row[:], EPS)

            x = xp.tile([P, DB, L], f32r, tag="x")
            x2 = xp.tile([P, DB, L], f32r, tag="x2")
            xmid = xp.tile([P, DB, L], f32r, tag="xmid")

            for m in range(DB):
                nc.sync.dma_start(x[:, m, 1:L], r(e1[m * P:(m + 1) * P, :]))
                nc.sync.dma_start(x2[:, m, 1:L], r(e2[m * P:(m + 1) * P, :]))
                nc.sync.dma_start(x[:, m, 0:1], r(cls_t[m * P:(m + 1) * P, :]))
                nc.sync.dma_start(x2[:, m, 0:1], r(cls_t[m * P:(m + 1) * P, :]))

            def layernorm(psp, pstag, rows, sqpool, sqtag, src, prms,
                          GSLOT, BSLOT, out_ap):
                """LN over features of src [P, DB, L] -> out_ap (may alias).

                Row stats accumulate across all q-chunks first, the rsqrt
                runs once per site, then the affine applies per chunk.
                """
                m_full = rows.tile([1, L], f32r, tag="lnm")
                s_full = rows.tile([1, L], f32, tag="lns")
                for (qo, qw) in QC:
                    sq = sqpool.tile([P, DB, 512], f32r, tag=sqtag)
                    nc.vector.tensor_mul(sq[:, :, :qw], src[:, :, qo:qo + qw],
                                         src[:, :, qo:qo + qw])
                    mp = psp.tile([P, 512], f32, tag=pstag)
                    for kt in range(DB):
                        mm(mp[0:1, :qw], ones_col[:, 0:1],
                           src[:, kt, qo:qo + qw], kt == 0, kt == DB - 1, qw)
                    sp = psp.tile([P, 512], f32, tag=pstag)
                    for kt in range(DB):
                        mm(sp[0:1, :qw], ones_col[:, 0:1], sq[:, kt, :qw],
                           kt == 0, kt == DB - 1, qw)
                    nc.vector.tensor_scalar_mul(m_full[:, qo:qo + qw],
                                                mp[0:1, :qw], 1.0 / D)
                    nc.vector.tensor_scalar_mul(s_full[:, qo:qo + qw],
                                                sp[0:1, :qw], 1.0 / D)
                # var = E[x^2] - mean^2 ; rstd = exp(-0.5*ln(var+eps))
                msq = rows.tile([1, L], f32, tag="lnt")
                nc.vector.tensor_mul(msq[:, :], m_full[:, :].bitcast(f32),
                                     m_full[:, :].bitcast(f32))
                nc.vector.tensor_sub(s_full[:, :], s_full[:, :], msq[:, :])
                nc.scalar.activation(s_full[:, :], s_full[:, :], AF.Ln,
                                     bias=eps_row[:])
                nc.vector.tensor_scalar_mul(s_full[:, :], s_full[:, :], -0.5)
                r_full = rows.tile([1, L], f32r, tag="lnr")
                nc.scalar.activation(r_full[:, :], s_full[:, :], AF.Exp)
                for (qo, qw) in QC:
                    mb = psp.tile([P, 512], f32, tag=pstag)
                    mm(mb[:, :qw], ones_row[0:1, :], m_full[0:1, qo:qo + qw],
                       True, True, qw)
                    rb = psp.tile([P, 512], f32, tag=pstag)
                    mm(rb[:, :qw], ones_row[0:1, :], r_full[0:1, qo:qo + qw],
                       True, True, qw)
                    sq = sqpool.tile([P, DB, 512], f32r, tag=sqtag)
                    for m in range(DB):
                        nc.vector.tensor_sub(sq[:, m, :qw],
                                             src[:, m, qo:qo + qw], mb[:, :qw])
                        nc.vector.tensor_mul(sq[:, m, :qw], sq[:, m, :qw],
                                             rb[:, :qw])
                        nc.scalar.activation(
                            out_ap[:, m, qo:qo + qw], sq[:, m, :qw],
                            AF.Identity,
                            bias=prms[:, BSLOT + m:BSLOT + m + 1],
                            scale=prms[:, GSLOT + m:GSLOT + m + 1])

            for l in range(NLAYER):
                prms = prm_pool.tile([P, NPARAM], f32, tag="prms")
                nc.sync.dma_start(prms[:], params_d[l, :, :])

                # =================== PHASE A: attention ===================
                with tc.tile_pool(name=f"wA{l}", bufs=1) as wp, \
                     tc.tile_pool(name=f"woA{l}", bufs=1) as wop, \
                     tc.tile_pool(name=f"kvA{l}", bufs=1) as ap1, \
                     tc.tile_pool(name=f"exA{l}", bufs=10) as exl, \
                     tc.tile_pool(name=f"sbA{l}", bufs=1) as ap3, \
                     tc.tile_pool(name=f"rwA{l}", bufs=1) as rows, \
                     tc.tile_pool(name=f"raA{l}", bufs=4) as rrows, \
                     tc.tile_pool(name=f"psA{l}", bufs=4, space="PSUM") as psA, \
                     tc.tile_pool(name=f"psC{l}", bufs=2, space="PSUM") as psC, \
                     tc.tile_pool(name=f"psW{l}", bufs=2, space="PSUM") as psW:

                    # ---- K projection (full L); softmax scale folded in ----
                    wk_sb = wp.tile([P, DB, D], f32r, tag="w")
                    nc.sync.dma_start(
                        wk_sb[:],
                        r(wkT[l, :, :].rearrange("(b p) n -> p b n", p=P)))
                    K_fm = ap1.tile([P, DB, L], f32r, tag="K")
                    for m in range(DB):
                        for (o, w) in QC:
                            kp = psA.tile([P, 512], f32, tag="psA")
                            for kt in range(DB):
                                mm(kp[:, :w],
                                   wk_sb[:, kt, m * P:(m + 1) * P],
                                   x2[:, kt, o:o + w],
                                   kt == 0, kt == DB - 1, w)
                            nc.vector.tensor_scalar(
                                out=K_fm[:, m, o:o + w], in0=kp[:, :w],
                                scalar1=prms[:, BK + m:BK + m + 1],
                                scalar2=SCALE, op0=AO.add, op1=AO.mult)

                    # ---- V projection (token-major, ones column at DH) ----
                    wv_sb = wp.tile([P, DB, D], f32r, tag="w")
                    nc.sync.dma_start(
                        wv_sb[:],
                        r(wvT[l, :, :].rearrange("(b p) n -> p b n", p=P)))
                    bvb = ap1.tile([P, D], f32, tag="bvb")
                    nc.sync.dma_start(
                        bvb[:],
                        bass.AP(tensor=bvrow_d, offset=l * D,
                                ap=[[0, P], [1, D]]))
                    V_tm = ap1.tile([P, LB, H, DH + 1], f32r, tag="V")
                    nc.vector.tensor_copy(
                        V_tm[:, :, :, DH],
                        ones_f32[:, 0:LB * H].rearrange("p (a b) -> p a b",
                                                        a=LB))
                    for mt in range(LB):
                        nrow = P if mt < LB - 1 else L - (LB - 1) * P
                        vp = psA.tile([P, 512], f32, tag="psA")
                        for kt in range(DB):
                            nc.tensor.matmul(
                                vp[:nrow, :D],
                                x2[:, kt, mt * P:mt * P + nrow],
                                wv_sb[:, kt, :],
                                start=(kt == 0), stop=(kt == DB - 1))
                        nc.vector.tensor_tensor(
                            out=V_tm[:nrow, mt, :, 0:DH],
                            in0=vp[:nrow, :D].rearrange("p (h c) -> p h c", h=H),
                            in1=bvb[:nrow, :].rearrange("p (h c) -> p h c", h=H),
                            op=AO.add)

                    # ---- wq & wo loads ----
                    wq_sb = wp.tile([P, DB, D], f32r, tag="w")
                    nc.sync.dma_start(
                        wq_sb[:],
                        r(wqT[l, :, :].rearrange("(b p) n -> p b n", p=P)))
                    wo_sb = wop.tile([DH, H, D], f32r, tag="wo")
                    nc.sync.dma_start(wo_sb[:], r(woTh[l, :, :, :]))

                    # ---- per q-chunk attention ----
                    for (qo, qw) in QC:
                        Q_fm = ap3.tile([P, DB, 512], f32r, tag="Q")
                        for m in range(DB):
                            qp = psA.tile([P, 512], f32, tag="psA")
                            for kt in range(DB):
                                mm(qp[:, :qw],
                                   wq_sb[:, kt, m * P:(m + 1) * P],
                                   x[:, kt, qo:qo + qw],
                                   kt == 0, kt == DB - 1, qw)
                            nc.vector.tensor_scalar_add(
                                Q_fm[:, m, :qw], qp[:, :qw],
                                prms[:, BQ + m:BQ + m + 1])

                        ctx_sb = ap3.tile([DH, H, 512], f32r, tag="ctx")

                        for h in range(H):
                            base, blk = (h % 2) * DH, h // 2
                            cp = psC.tile([DH + 1, 512], f32, tag="psC")
                            ets = []
                            for kt in range(LB):
                                nrow = P if kt < LB - 1 else L - (LB - 1) * P
                                sp = psA.tile([P, 512], f32, tag="psA")
                                mm(sp[:nrow, :qw],
                                   K_fm[base:base + DH, blk,
                                        kt * P:kt * P + nrow],
                                   Q_fm[base:base + DH, blk, :qw],
                                   True, True, qw)
                                et = exl.tile([P, 512], f32r, tag="exp")
                                nc.scalar.activation(et[:nrow, :qw],
                                                     sp[:nrow, :qw], AF.Exp)
                                ets.append(et)
                            for kt in range(LB):
                                nrow = P if kt < LB - 1 else L - (LB - 1) * P
                                mm(cp[:, :qw],
                                   V_tm[:nrow, kt, h, :],
                                   ets[kt][:nrow, :qw],
                                   kt == 0, kt == LB - 1, qw)
                            # normalize by the ones-column denominators
                            rrow = rrows.tile([1, 512], f32, tag="row")
                            nc.vector.reciprocal(rrow[:, :qw],
                                                 cp[DH:DH + 1, :qw])
                            rrowr = rrows.tile([1, 512], f32r, tag="row")
                            nc.vector.tensor_copy(rrowr[:, :qw], rrow[:, :qw])
                            rb = psA.tile([P, 512], f32, tag="psA")
                            mm(rb[:DH, :qw], ones_row[0:1, 0:DH],
                               rrowr[0:1, :qw], True, True, qw)
                            nc.vector.tensor_copy(ctx_sb[:, h, :qw],
                                                  cp[:DH, :qw])
                            nc.vector.tensor_tensor(
                                out=ctx_sb[:, h, :qw], in0=ctx_sb[:, h, :qw],
                                in1=rb[:DH, :qw], op=AO.mult)

                        # ---- output projection + bo + residual -> xmid ----
                        for m in range(DB):
                            op_ = psW.tile([P, 512], f32, tag="psW")
                            for h in range(H):
                                mm(op_[:, :qw],
                                   wo_sb[:, h, m * P:(m + 1) * P],
                                   ctx_sb[:, h, :qw],
                                   h == 0, h == H - 1, qw)
                            nc.vector.scalar_tensor_tensor(
                                out=xmid[:, m, qo:qo + qw], in0=op_[:, :qw],
                                scalar=prms[:, BO + m:BO + m + 1],
                                in1=x[:, m, qo:qo + qw],
                                op0=AO.add, op1=AO.add)

                    # ---- LN1 (in place on xmid) ----
                    layernorm(psA, "psA", rows, ap3, "sq", xmid, prms,
                              LN1G, LN1B, xmid)

                # =================== PHASE B: FFN ===================
                with tc.tile_pool(name=f"wB{l}", bufs=1) as fwp, \
                     tc.tile_pool(name=f"hB{l}", bufs=1) as fhp, \
                     tc.tile_pool(name=f"sqB{l}", bufs=1) as fsq, \
                     tc.tile_pool(name=f"rwB{l}", bufs=1) as rowsB, \
                     tc.tile_pool(name=f"psH{l}", bufs=4, space="PSUM") as psH, \
                     tc.tile_pool(name=f"psF{l}", bufs=2, space="PSUM") as psF:

                    w1_sb = fwp.tile([P, DB, DFF], f32r, tag="w1")
                    nc.sync.dma_start(
                        w1_sb[:],
                        r(w1T[l, :, :].rearrange("(b p) n -> p b n", p=P)))
                    w2_sb = fwp.tile([P, FB, D], f32r, tag="w2")
                    nc.sync.dma_start(
                        w2_sb[:],
                        r(w2T[l, :, :].rearrange("(b p) n -> p b n", p=P)))

                    for (qo, qw) in QC:
                        h_sb = fhp.tile([P, FB, 512], f32r, tag="h")
                        for mf in range(FB):
                            hp = psH.tile([P, 512], f32, tag="psH")
                            for kt in range(DB):
                                mm(hp[:, :qw],
                                   w1_sb[:, kt, mf * P:(mf + 1) * P],
                                   xmid[:, kt, qo:qo + qw],
                                   kt == 0, kt == DB - 1, qw)
                            nc.scalar.activation(
                                h_sb[:, mf, :qw], hp[:, :qw], AF.Gelu,
                                bias=prms[:, B1S + mf:B1S + mf + 1])
                        for m in range(DB):
                            fp = psF.tile([P, 512], f32, tag="psF")
                            for kt in range(FB):
                                mm(fp[:, :qw],
                                   w2_sb[:, kt, m * P:(m + 1) * P],
                                   h_sb[:, kt, :qw],
                                   kt == 0, kt == FB - 1, qw)
                            nc.vector.scalar_tensor_tensor(
                                out=x[:, m, qo:qo + qw], in0=fp[:, :qw],
                                scalar=prms[:, B2S + m:B2S + m + 1],
                                in1=xmid[:, m, qo:qo + qw],
                                op0=AO.add, op1=AO.add)

                    # ---- LN2 (in place on x) ----
                    layernorm(psH, "psH", rowsB, fsq, "sq", x, prms,
                              LN2G, LN2B, x)

            # =================== transpose x -> out ===================
            with tc.tile_pool(name="psT", bufs=4, space="PSUM") as psT, \
                 tc.tile_pool(name="sbT", bufs=4) as sbT:
                for mt in range(LB):
                    nrow = P if mt < LB - 1 else L - (LB - 1) * P
                    for m in range(DB):
                        tp = psT.tile([P, P], f32, tag="psT")
                        nc.tensor.transpose(
                            tp[:nrow, :],
                            x[:, m, mt * P:mt * P + nrow].bitcast(f32),
                            ident[:])
                        ts = sbT.tile([P, P], f32, tag="sbT")
                        nc.vector.tensor_copy(ts[:nrow, :], tp[:nrow, :])
                        nc.sync.dma_start(
                            out_d[mt * P:mt * P + nrow, m * P:(m + 1) * P],
                            ts[:nrow, :])

    nc.compile()
    return nc


# ---------------------------------------------------------------- host side
def _prep_inputs(inputs):
    f = np.float32
    Wq, Wk, Wv, Wo = inputs["Wq"], inputs["Wk"], inputs["Wv"], inputs["Wo"]
    W1, W2 = inputs["W1"], inputs["W2"]

    wqT = np.ascontiguousarray(np.transpose(np.asarray(Wq, f), (0, 2, 1)))
    wkT = np.ascontiguousarray(np.transpose(np.asarray(Wk, f), (0, 2, 1)))
    wvT = np.ascontiguousarray(np.transpose(np.asarray(Wv, f), (0, 2, 1)))
    w1T = np.ascontiguousarray(np.transpose(np.asarray(W1, f), (0, 2, 1)))
    w2T = np.ascontiguousarray(np.transpose(np.asarray(W2, f), (0, 2, 1)))
    woTh = np.ascontiguousarray(
        np.transpose(np.asarray(Wo, f).reshape(NLAYER, D, H, DH), (0, 3, 2, 1)))

    def col(v):  # [NLAYER, D] -> [NLAYER, P, DB]
        return np.transpose(np.asarray(v, f).reshape(NLAYER, DB, P), (0, 2, 1))

    params = np.zeros((NLAYER, P, NPARAM), f)
    params[:, :, BQ:BQ + DB] = col(inputs["bq"])
    params[:, :, BK:BK + DB] = col(inputs["bk"])
    params[:, :, BV:BV + DB] = col(inputs["bv"])
    params[:, :, BO:BO + DB] = col(inputs["bo"])
    params[:, :, B2S:B2S + DB] = col(inputs["b2"])
    params[:, :, LN1G:LN1G + DB] = col(inputs["ln1_g"])
    params[:, :, LN1B:LN1B + DB] = col(inputs["ln1_b"])
    params[:, :, LN2G:LN2G + DB] = col(inputs["ln2_g"])
    params[:, :, LN2B:LN2B + DB] = col(inputs["ln2_b"])
    params[:, :, B1S:B1S + FB] = np.transpose(
        np.asarray(inputs["b1"], f).reshape(NLAYER, FB, P), (0, 2, 1))

    shared = {
        "cls": np.ascontiguousarray(
            np.asarray(inputs["cls_token"], f).reshape(D, 1)),
        "wqT": wqT, "wkT": wkT, "wvT": wvT, "woTh": woTh,
        "w1T": w1T, "w2T": w2T, "params": params,
        "bvrow": np.ascontiguousarray(np.asarray(inputs["bv"], f)),
    }
    e1 = np.asarray(inputs["embed1"], f)
    e2 = np.asarray(inputs["embed2"], f)
    in_maps = []
    for b in range(N_CORES):
        m = dict(shared)
        m["e1"] = np.ascontiguousarray(e1[b])
        m["e2"] = np.ascontiguousarray(e2[b])
        in_maps.append(m)
    return in_maps


def _run(inputs, trace=False, **kw):
    from concourse.bass_utils import run_bass_kernel_spmd

    if "nc" not in _CACHE:
        _CACHE["nc"] = _build_nc()
    nc = _CACHE["nc"]
    in_maps = _prep_inputs(inputs)
    res = run_bass_kernel_spmd(nc, in_maps, list(range(N_CORES)), trace=trace,
                               **kw)
    out = np.stack([res.results[b]["out"] for b in range(N_CORES)], axis=0)
    return out.astype(np.float32), res


def kernel(**inputs):
    out, _ = _run(inputs, trace=False)
    return out


# revision 18
# speedup vs baseline: 1.3088x; 1.1847x over previous
"""Trainium2 Bass kernel for a 2-layer cross-attention dense transformer.

Sharding: data-parallel over batch — B=8 batch elements, one per NeuronCore.
Each core runs the full 2-layer transformer on its batch element; weights are
replicated. No collectives.

Per-core layout choices:
  - Activations are kept FEATURE-major in SBUF: x_fm[p, blk, l] = x[blk*128+p, l].
    All matmuls contract over features, so no transposes are needed anywhere.
  - Attention scores are computed transposed (scores_T[k_pos, q]); softmax
    denominators are obtained by appending a ones-column to the V tile in the
    attn@V matmul (M=65 per head), then dividing at PSUM eviction.
  - No max-subtraction in softmax (scores are O(5); exp is safe in fp32).
  - Matmuls run in float32r (single-pass fp32; ~TF32 input rounding). PSUM fp32.
  - LayerNorm partition sums via ones-vector matmuls, accumulated to full-L
    row tiles; rsqrt = exp(-0.5*ln(v+eps)) ONCE per layer-norm site so the
    ACT table only reloads a few times per layer.
"""

import numpy as np

# ---------------------------------------------------------------- constants
B, D, L0 = 8, 512, 1024
L = L0 + 1            # 1025 tokens (cls + 1024)
H, DH = 8, 64
DFF = 2048
NLAYER = 2
EPS = 1e-6
SCALE = 1.0 / (DH ** 0.5)

P = 128
DB = D // P           # 4 feature blocks
FB = DFF // P         # 16 dff blocks
LB = 9                # l-tiles over padded length (8 full + 2 rows)

N_CORES = 8


def _chunks(total, width):
    out, o = [], 0
    while o < total:
        w = min(width, total - o)
        out.append((o, w))
        o += w
    return out


LP = 1026                  # padded length: 3 even chunks of 342, no edges
QC = [(0, 342), (342, 342), (684, 342)]

# params tile slot indices (free-dim column j of the [128, NPARAM] tile)
BQ, BK, BV, BO, B2S = 0, 4, 8, 12, 16
LN1G, LN1B, LN2G, LN2B = 20, 24, 28, 32
B1S = 36
NPARAM = 52

_CACHE = {}


# ---------------------------------------------------------------- bass build
def _build_nc():
    import concourse.bass as bass
    import concourse.bacc as bacc
    import concourse.tile as tile
    from concourse import mybir
    from concourse.masks import make_identity

    f32 = mybir.dt.float32
    f32r = mybir.dt.float32r
    AO = mybir.AluOpType
    AF = mybir.ActivationFunctionType

    nc = bacc.Bacc("TRN2", target_bir_lowering=False, debug=False)

    # ---- DRAM I/O (per core) ----
    e1 = nc.dram_tensor("e1", [D, L0], f32, kind="ExternalInput")
    e2 = nc.dram_tensor("e2", [D, L0], f32, kind="ExternalInput")
    cls_t = nc.dram_tensor("cls", [D, 1], f32, kind="ExternalInput")
    wqT = nc.dram_tensor("wqT", [NLAYER, D, D], f32, kind="ExternalInput")
    wkT = nc.dram_tensor("wkT", [NLAYER, D, D], f32, kind="ExternalInput")
    wvT = nc.dram_tensor("wvT", [NLAYER, D, D], f32, kind="ExternalInput")
    woTh = nc.dram_tensor("woTh", [NLAYER, DH, H, D], f32, kind="ExternalInput")
    w1T = nc.dram_tensor("w1T", [NLAYER, D, DFF], f32, kind="ExternalInput")
    w2T = nc.dram_tensor("w2T", [NLAYER, DFF, D], f32, kind="ExternalInput")
    params_d = nc.dram_tensor("params", [NLAYER, P, NPARAM], f32,
                              kind="ExternalInput")
    bvrow_d = nc.dram_tensor("bvrow", [NLAYER, D], f32, kind="ExternalInput")
    out_d = nc.dram_tensor("out", [L, D], f32, kind="ExternalOutput")

    def r(ap):
        return ap.bitcast(f32r)

    def mm(out, lhsT, rhs, start, stop, n):
        # fp32r matmuls fail ISA checks for 1-wide moving operands; fall back
        # to plain fp32 there (edge chunks only).
        if n == 1:
            nc.tensor.matmul(out, lhsT.bitcast(f32), rhs.bitcast(f32),
                             start=start, stop=stop)
        else:
            nc.tensor.matmul(out, lhsT, rhs, start=start, stop=stop)

    with tile.TileContext(nc) as tc:
        with tc.tile_pool(name="persist", bufs=1) as pp, \
             tc.tile_pool(name="xpool", bufs=1) as xp, \
             tc.tile_pool(name="parms", bufs=2) as prm_pool:

            ones_f32 = pp.tile([P, P], f32)
            nc.vector.memset(ones_f32[:], 1.0)
            ones_col = pp.tile([P, 1], f32r)
            nc.vector.tensor_copy(ones_col[:], ones_f32[:, 0:1])
            ones_row = pp.tile([1, P], f32r)
            nc.vector.tensor_copy(ones_row[:], ones_f32[0:1, :])
            ident = pp.tile([P, P], f32)
            make_identity(nc, ident[:])
            eps_row = pp.tile([1, 1], f32)
            nc.vector.memset(eps_row[:], EPS)

            x = xp.tile([P, DB, LP], f32r, tag="x")
            x2 = xp.tile([P, DB, LP], f32r, tag="x2")
            xmid = xp.tile([P, DB, LP], f32r, tag="xmid")

            for m in range(DB):
                nc.sync.dma_start(x[:, m, 1:L], r(e1[m * P:(m + 1) * P, :]))
                nc.sync.dma_start(x2[:, m, 1:L], r(e2[m * P:(m + 1) * P, :]))
                nc.sync.dma_start(x[:, m, 0:1], r(cls_t[m * P:(m + 1) * P, :]))
                nc.sync.dma_start(x2[:, m, 0:1], r(cls_t[m * P:(m + 1) * P, :]))
            # zero the single pad token column (keeps everything finite)
            nc.vector.tensor_scalar_mul(x[:, :, L], ones_f32[:, 0:DB], 0.0)
            nc.vector.tensor_scalar_mul(x2[:, :, L], ones_f32[:, 0:DB], 0.0)

            def layernorm(psp, pstag, rows, sqpool, sqtag, src, prms,
                          GSLOT, BSLOT, out_ap):
                """LN over features of src [P, DB, L] -> out_ap (may alias).

                Row stats accumulate across all q-chunks first, the rsqrt
                runs once per site, then the affine applies per chunk.
                """
                m_full = rows.tile([1, LP], f32r, tag="lnm")
                s_full = rows.tile([1, LP], f32, tag="lns")
                for (qo, qw) in QC:
                    sq = sqpool.tile([P, DB, 512], f32r, tag=sqtag)
                    nc.vector.tensor_mul(sq[:, :, :qw], src[:, :, qo:qo + qw],
                                         src[:, :, qo:qo + qw])
                    mp = psp.tile([P, 512], f32, tag=pstag)
                    for kt in range(DB):
                        mm(mp[0:1, :qw], ones_col[:, 0:1],
                           src[:, kt, qo:qo + qw], kt == 0, kt == DB - 1, qw)
                    sp = psp.tile([P, 512], f32, tag=pstag)
                    for kt in range(DB):
                        mm(sp[0:1, :qw], ones_col[:, 0:1], sq[:, kt, :qw],
                           kt == 0, kt == DB - 1, qw)
                    nc.vector.tensor_scalar_mul(m_full[:, qo:qo + qw],
                                                mp[0:1, :qw], 1.0 / D)
                    nc.vector.tensor_scalar_mul(s_full[:, qo:qo + qw],
                                                sp[0:1, :qw], 1.0 / D)
                # var = E[x^2] - mean^2 ; rstd = exp(-0.5*ln(var+eps))
                msq = rows.tile([1, LP], f32, tag="lnt")
                nc.vector.tensor_mul(msq[:, :], m_full[:, :].bitcast(f32),
                                     m_full[:, :].bitcast(f32))
                nc.vector.tensor_sub(s_full[:, :], s_full[:, :], msq[:, :])
                nc.scalar.activation(s_full[:, :], s_full[:, :], AF.Ln,
                                     bias=eps_row[:])
                nc.vector.tensor_scalar_mul(s_full[:, :], s_full[:, :], -0.5)
                r_full = rows.tile([1, LP], f32r, tag="lnr")
                nc.scalar.activation(r_full[:, :], s_full[:, :], AF.Exp)
                for (qo, qw) in QC:
                    mb = psp.tile([P, 512], f32, tag=pstag)
                    mm(mb[:, :qw], ones_row[0:1, :], m_full[0:1, qo:qo + qw],
                       True, True, qw)
                    rb = psp.tile([P, 512], f32, tag=pstag)
                    mm(rb[:, :qw], ones_row[0:1, :], r_full[0:1, qo:qo + qw],
                       True, True, qw)
                    sq = sqpool.tile([P, DB, 512], f32r, tag=sqtag)
                    for m in range(DB):
                        nc.vector.tensor_sub(sq[:, m, :qw],
                                             src[:, m, qo:qo + qw], mb[:, :qw])
                        nc.vector.tensor_mul(sq[:, m, :qw], sq[:, m, :qw],
                                             rb[:, :qw])
                        nc.scalar.activation(
                            out_ap[:, m, qo:qo + qw], sq[:, m, :qw],
                            AF.Identity,
                            bias=prms[:, BSLOT + m:BSLOT + m + 1],
                            scale=prms[:, GSLOT + m:GSLOT + m + 1])

            for l in range(NLAYER):
                prms = prm_pool.tile([P, NPARAM], f32, tag="prms")
                nc.sync.dma_start(prms[:], params_d[l, :, :])

                # =================== PHASE A: attention ===================
                with tc.tile_pool(name=f"wA{l}", bufs=1) as wp, \
                     tc.tile_pool(name=f"woA{l}", bufs=1) as wop, \
                     tc.tile_pool(name=f"kvA{l}", bufs=1) as ap1, \
                     tc.tile_pool(name=f"exA{l}", bufs=10) as exl, \
                     tc.tile_pool(name=f"sbA{l}", bufs=1) as ap3, \
                     tc.tile_pool(name=f"rwA{l}", bufs=1) as rows, \
                     tc.tile_pool(name=f"raA{l}", bufs=4) as rrows, \
                     tc.tile_pool(name=f"psA{l}", bufs=4, space="PSUM") as psA, \
                     tc.tile_pool(name=f"psC{l}", bufs=2, space="PSUM") as psC, \
                     tc.tile_pool(name=f"psW{l}", bufs=2, space="PSUM") as psW:

                    # ---- K projection (full L); softmax scale folded in ----
                    wk_sb = wp.tile([P, DB, D], f32r, tag="w")
                    nc.sync.dma_start(
                        wk_sb[:],
                        r(wkT[l, :, :].rearrange("(b p) n -> p b n", p=P)))
                    K_fm = ap1.tile([P, DB, LP], f32r, tag="K")
                    for m in range(DB):
                        for (o, w) in QC:
                            kp = psA.tile([P, 512], f32, tag="psA")
                            for kt in range(DB):
                                mm(kp[:, :w],
                                   wk_sb[:, kt, m * P:(m + 1) * P],
                                   x2[:, kt, o:o + w],
                                   kt == 0, kt == DB - 1, w)
                            nc.vector.tensor_scalar(
                                out=K_fm[:, m, o:o + w], in0=kp[:, :w],
                                scalar1=prms[:, BK + m:BK + m + 1],
                                scalar2=SCALE, op0=AO.add, op1=AO.mult)

                    # ---- V projection (token-major, ones column at DH) ----
                    wv_sb = wp.tile([P, DB, D], f32r, tag="w")
                    nc.sync.dma_start(
                        wv_sb[:],
                        r(wvT[l, :, :].rearrange("(b p) n -> p b n", p=P)))
                    bvb = ap1.tile([P, D], f32, tag="bvb")
                    nc.sync.dma_start(
                        bvb[:],
                        bass.AP(tensor=bvrow_d, offset=l * D,
                                ap=[[0, P], [1, D]]))
                    V_tm = ap1.tile([P, LB, H, DH + 1], f32r, tag="V")
                    nc.vector.tensor_copy(
                        V_tm[:, :, :, DH],
                        ones_f32[:, 0:LB * H].rearrange("p (a b) -> p a b",
                                                        a=LB))
                    for mt in range(LB):
                        nrow = P if mt < LB - 1 else L - (LB - 1) * P
                        vp = psA.tile([P, 512], f32, tag="psA")
                        for kt in range(DB):
                            nc.tensor.matmul(
                                vp[:nrow, :D],
                                x2[:, kt, mt * P:mt * P + nrow],
                                wv_sb[:, kt, :],
                                start=(kt == 0), stop=(kt == DB - 1))
                        nc.vector.tensor_tensor(
                            out=V_tm[:nrow, mt, :, 0:DH],
                            in0=vp[:nrow, :D].rearrange("p (h c) -> p h c", h=H),
                            in1=bvb[:nrow, :].rearrange("p (h c) -> p h c", h=H),
                            op=AO.add)

                    # ---- wq & wo loads ----
                    wq_sb = wp.tile([P, DB, D], f32r, tag="w")
                    nc.sync.dma_start(
                        wq_sb[:],
                        r(wqT[l, :, :].rearrange("(b p) n -> p b n", p=P)))
                    wo_sb = wop.tile([DH, H, D], f32r, tag="wo")
                    nc.sync.dma_start(wo_sb[:], r(woTh[l, :, :, :]))

                    # ---- per q-chunk attention ----
                    for (qo, qw) in QC:
                        Q_fm = ap3.tile([P, DB, 512], f32r, tag="Q")
                        for m in range(DB):
                            qp = psA.tile([P, 512], f32, tag="psA")
                            for kt in range(DB):
                                mm(qp[:, :qw],
                                   wq_sb[:, kt, m * P:(m + 1) * P],
                                   x[:, kt, qo:qo + qw],
                                   kt == 0, kt == DB - 1, qw)
                            nc.vector.tensor_scalar_add(
                                Q_fm[:, m, :qw], qp[:, :qw],
                                prms[:, BQ + m:BQ + m + 1])

                        ctx_sb = ap3.tile([DH, H, 512], f32r, tag="ctx")

                        for h in range(H):
                            base, blk = (h % 2) * DH, h // 2
                            cp = psC.tile([DH + 1, 512], f32, tag="psC")
                            ets = []
                            for kt in range(LB):
                                nrow = P if kt < LB - 1 else L - (LB - 1) * P
                                sp = psA.tile([P, 512], f32, tag="psA")
                                mm(sp[:nrow, :qw],
                                   K_fm[base:base + DH, blk,
                                        kt * P:kt * P + nrow],
                                   Q_fm[base:base + DH, blk, :qw],
                                   True, True, qw)
                                et = exl.tile([P, 512], f32r, tag="exp")
                                nc.scalar.activation(et[:nrow, :qw],
                                                     sp[:nrow, :qw], AF.Exp)
                                ets.append(et)
                            for kt in range(LB):
                                nrow = P if kt < LB - 1 else L - (LB - 1) * P
                                mm(cp[:, :qw],
                                   V_tm[:nrow, kt, h, :],
                                   ets[kt][:nrow, :qw],
                                   kt == 0, kt == LB - 1, qw)
                            # normalize by the ones-column denominators
                            rrow = rrows.tile([1, 512], f32, tag="row")
                            nc.vector.reciprocal(rrow[:, :qw],
                                                 cp[DH:DH + 1, :qw])
                            rrowr = rrows.tile([1, 512], f32r, tag="row")
                            nc.vector.tensor_copy(rrowr[:, :qw], rrow[:, :qw])
                            rb = psA.tile([P, 512], f32, tag="psA")
                            mm(rb[:DH, :qw], ones_row[0:1, 0:DH],
                               rrowr[0:1, :qw], True, True, qw)
                            nc.vector.tensor_copy(ctx_sb[:, h, :qw],
                                                  cp[:DH, :qw])
                            nc.vector.tensor_tensor(
                                out=ctx_sb[:, h, :qw], in0=ctx_sb[:, h, :qw],
                                in1=rb[:DH, :qw], op=AO.mult)

                        # ---- output projection + bo + residual -> xmid ----
                        for m in range(DB):
                            op_ = psW.tile([P, 512], f32, tag="psW")
                            for h in range(H):
                                mm(op_[:, :qw],
                                   wo_sb[:, h, m * P:(m + 1) * P],
                                   ctx_sb[:, h, :qw],
                                   h == 0, h == H - 1, qw)
                            nc.vector.scalar_tensor_tensor(
                                out=xmid[:, m, qo:qo + qw], in0=op_[:, :qw],
                                scalar=prms[:, BO + m:BO + m + 1],
                                in1=x[:, m, qo:qo + qw],
                                op0=AO.add, op1=AO.add)

                    # ---- LN1 (in place on xmid) ----
                    layernorm(psA, "psA", rows, ap3, "sq", xmid, prms,
                              LN1G, LN1B, xmid)

                # =================== PHASE B: FFN ===================
                with tc.tile_pool(name=f"wB{l}", bufs=1) as fwp, \
                     tc.tile_pool(name=f"hB{l}", bufs=1) as fhp, \
                     tc.tile_pool(name=f"sqB{l}", bufs=1) as fsq, \
                     tc.tile_pool(name=f"rwB{l}", bufs=1) as rowsB, \
                     tc.tile_pool(name=f"psH{l}", bufs=4, space="PSUM") as psH, \
                     tc.tile_pool(name=f"psF{l}", bufs=2, space="PSUM") as psF:

                    w1_sb = fwp.tile([P, DB, DFF], f32r, tag="w1")
                    nc.sync.dma_start(
                        w1_sb[:],
                        r(w1T[l, :, :].rearrange("(b p) n -> p b n", p=P)))
                    w2_sb = fwp.tile([P, FB, D], f32r, tag="w2")
                    nc.sync.dma_start(
                        w2_sb[:],
                        r(w2T[l, :, :].rearrange("(b p) n -> p b n", p=P)))

                    for (qo, qw) in QC:
                        h_sb = fhp.tile([P, FB, 512], f32r, tag="h")
                        for mf in range(FB):
                            hp = psH.tile([P, 512], f32, tag="psH")
                            for kt in range(DB):
                                mm(hp[:, :qw],
                                   w1_sb[:, kt, mf * P:(mf + 1) * P],
                                   xmid[:, kt, qo:qo + qw],
                                   kt == 0, kt == DB - 1, qw)
                            nc.scalar.activation(
                                h_sb[:, mf, :qw], hp[:, :qw], AF.Gelu,
                                bias=prms[:, B1S + mf:B1S + mf + 1])
                        for m in range(DB):
                            fp = psF.tile([P, 512], f32, tag="psF")
                            for kt in range(FB):
                                mm(fp[:, :qw],
                                   w2_sb[:, kt, m * P:(m + 1) * P],
                                   h_sb[:, kt, :qw],
                                   kt == 0, kt == FB - 1, qw)
                            nc.vector.scalar_tensor_tensor(
                                out=x[:, m, qo:qo + qw], in0=fp[:, :qw],
                                scalar=prms[:, B2S + m:B2S + m + 1],
                                in1=xmid[:, m, qo:qo + qw],
                                op0=AO.add, op1=AO.add)

                    # ---- LN2 (in place on x) ----
                    layernorm(psH, "psH", rowsB, fsq, "sq", x, prms,
                              LN2G, LN2B, x)

            # =================== transpose x -> out ===================
            with tc.tile_pool(name="psT", bufs=4, space="PSUM") as psT, \
                 tc.tile_pool(name="sbT", bufs=4) as sbT:
                for mt in range(LB):
                    nrow = P if mt < LB - 1 else L - (LB - 1) * P
                    for m in range(DB):
                        tp = psT.tile([P, P], f32, tag="psT")
                        nc.tensor.transpose(
                            tp[:nrow, :],
                            x[:, m, mt * P:mt * P + nrow].bitcast(f32),
                            ident[:])
                        ts = sbT.tile([P, P], f32, tag="sbT")
                        nc.vector.tensor_copy(ts[:nrow, :], tp[:nrow, :])
                        nc.sync.dma_start(
                            out_d[mt * P:mt * P + nrow, m * P:(m + 1) * P],
                            ts[:nrow, :])

    nc.compile()
    return nc


# ---------------------------------------------------------------- host side
def _prep_inputs(inputs):
    f = np.float32
    Wq, Wk, Wv, Wo = inputs["Wq"], inputs["Wk"], inputs["Wv"], inputs["Wo"]
    W1, W2 = inputs["W1"], inputs["W2"]

    wqT = np.ascontiguousarray(np.transpose(np.asarray(Wq, f), (0, 2, 1)))
    wkT = np.ascontiguousarray(np.transpose(np.asarray(Wk, f), (0, 2, 1)))
    wvT = np.ascontiguousarray(np.transpose(np.asarray(Wv, f), (0, 2, 1)))
    w1T = np.ascontiguousarray(np.transpose(np.asarray(W1, f), (0, 2, 1)))
    w2T = np.ascontiguousarray(np.transpose(np.asarray(W2, f), (0, 2, 1)))
    woTh = np.ascontiguousarray(
        np.transpose(np.asarray(Wo, f).reshape(NLAYER, D, H, DH), (0, 3, 2, 1)))

    def col(v):  # [NLAYER, D] -> [NLAYER, P, DB]
        return np.transpose(np.asarray(v, f).reshape(NLAYER, DB, P), (0, 2, 1))

    params = np.zeros((NLAYER, P, NPARAM), f)
    params[:, :, BQ:BQ + DB] = col(inputs["bq"])
    params[:, :, BK:BK + DB] = col(inputs["bk"])
    params[:, :, BV:BV + DB] = col(inputs["bv"])
    params[:, :, BO:BO + DB] = col(inputs["bo"])
    params[:, :, B2S:B2S + DB] = col(inputs["b2"])
    params[:, :, LN1G:LN1G + DB] = col(inputs["ln1_g"])
    params[:, :, LN1B:LN1B + DB] = col(inputs["ln1_b"])
    params[:, :, LN2G:LN2G + DB] = col(inputs["ln2_g"])
    params[:, :, LN2B:LN2B + DB] = col(inputs["ln2_b"])
    params[:, :, B1S:B1S + FB] = np.transpose(
        np.asarray(inputs["b1"], f).reshape(NLAYER, FB, P), (0, 2, 1))

    shared = {
        "cls": np.ascontiguousarray(
            np.asarray(inputs["cls_token"], f).reshape(D, 1)),
        "wqT": wqT, "wkT": wkT, "wvT": wvT, "woTh": woTh,
        "w1T": w1T, "w2T": w2T, "params": params,
        "bvrow": np.ascontiguousarray(np.asarray(inputs["bv"], f)),
    }
    e1 = np.asarray(inputs["embed1"], f)
    e2 = np.asarray(inputs["embed2"], f)
    in_maps = []
    for b in range(N_CORES):
        m = dict(shared)
        m["e1"] = np.ascontiguousarray(e1[b])
        m["e2"] = np.ascontiguousarray(e2[b])
        in_maps.append(m)
    return in_maps


def _run(inputs, trace=False, **kw):
    from concourse.bass_utils import run_bass_kernel_spmd

    if "nc" not in _CACHE:
        _CACHE["nc"] = _build_nc()
    nc = _CACHE["nc"]
    in_maps = _prep_inputs(inputs)
    res = run_bass_kernel_spmd(nc, in_maps, list(range(N_CORES)), trace=trace,
                               **kw)
    out = np.stack([res.results[b]["out"] for b in range(N_CORES)], axis=0)
    return out.astype(np.float32), res


def kernel(**inputs):
    out, _ = _run(inputs, trace=False)
    return out
